# revision 5
# baseline (speedup 1.0000x reference)
"""Multi-head attention (B=4, S=2048, D=1024, 16 heads x 64) on 8 trn2 cores.

Sharding: core c handles batch b = c//2 and head-group hg = c%2 (8 heads each,
i.e. columns hg*512:(hg+1)*512 of Wq/Wk/Wv and rows of Wo).  Each core returns
a partial output [S, D]; the host sums the two partials per batch and adds bo.

Per-core kernel (everything "T" = feature-on-partition layout):
  phase 1: QT = (Wq.T @ Xq.T)+bq  [512, S],  KT likewise but scaled by
           KSC = 16*log2(e) (host-folded into wk/bk) so score PSUM values are
           2^7 * log2(e^(s/8)).  V = Xv@Wv+bv [S, 512] bf16.
  phase 2: per head-pair hp and 512-col query chunk c:
           scoresT[sk,q] = KT_h.T @ QT_h via row-paired (64+64) matmuls.
           probsT = e^(s/8)/sqrt(2) in bf16, computed on ScalarE (Exp with
           scale=1/(128 log2 e), bias=-ln2/2) for most key tiles and on a
           custom DVE op (Schraudolph exp2-bits with quadratic mantissa
           correction, int16 output bitcast to bf16) for the rest - splitting
           the exp work across both engines.  The 1/sqrt(2) factor is
           softmax-invariant.
           zT pair [128, 512] = V_h.T @ probsT via column-paired (64+64)
           matmuls accumulated over the 16 key tiles (full PE-array rate).
           Softmax denominators: probsT tiles are tree-summed over sk into
           denpart [128, 512] per head (DVE/GpSimd bf16 adds), then one
           column-paired ones-vector matmul reduces denpart over keys.
           Normalize: reciprocal (DVE) -> partition-broadcast (GpSimd) ->
           multiply during PSUM eviction -> ZT [512, S] fp32.
  phase 3: out = Z @ Wo, natural [S, D] layout, DMA'd to DRAM.
"""

import numpy as np

import concourse.bass as bass
import concourse.tile as tile
from concourse import bacc, mybir
from concourse.bass_utils import run_bass_kernel_spmd

F32 = mybir.dt.float32
F32R = mybir.dt.float32r
BF16 = mybir.dt.bfloat16
I16 = mybir.dt.int16
ACT = mybir.ActivationFunctionType

D = 1024          # d_model
HH = 512          # heads-per-core * head_dim = 8 * 64
HD = 64           # head dim
NHL = 8           # heads per core
B, S_FULL = 4, 2048
N_CORES = 8

LOG2E = float(np.log2(np.e))
KSC = 16.0 * LOG2E          # host-folded into wk/bk: score PSUM = 2^7*u
EXP_SCALE = 1.0 / (128.0 * LOG2E)   # ScalarE: exp(s'*scale + bias)
EXP_BIAS = float(-0.5 * np.log(2.0))

# custom DVE exp2-bits constants (quadratic fit of 2^(g+.5)-1 on [-.5,.5])
A0, A1, A2 = 0.41473428, 0.99231215, 0.33593699
MAGIC = float(1.5 * 2**30)
C3VAL = 128.0 * (126.0 + A0)

# exp-work split: sk-tile groups (of 3, last is 1) routed to the DVE op
# instead of ScalarE.  Groups: [0:3],[3:6],[6:9],[9:12],[12:15],[15:16].
DVE_EXP_GROUPS = frozenset({1, 5})


def _register_exp_op():
    import concourse.dve_ops as dve_ops
    from concourse.dve_spec import Spec, Src0, C0, C1, C2, C3, lower
    from concourse.dve_uop import DveOpSpec
    from concourse.dve_ops import DveOp, _spill_c3_to_src1

    name = "MHA_EXP2_BITS16"
    for o in dve_ops.OPS:
        if o.name == name:
            return o
    t = Src0 + C0
    r = t - C0
    fp = Src0 - r
    body = ((fp * C1 + C2) * fp + r) + C3

    def ref(in0, in1, s0, s1, imm2):
        t = (in0.astype(np.float32) + s0).astype(np.float32)
        r = (t - s0).astype(np.float32)
        fp = (in0.astype(np.float32) - r).astype(np.float32)
        c3 = np.broadcast_to(in1[:, :1], in0.shape).astype(np.float32)
        return ((fp * s1 + imm2) * fp + r) + c3

    spec = Spec(body=_spill_c3_to_src1(body), reference=ref)
    shas = {}
    for ver in ("v3", "v4"):
        shas[ver] = DveOpSpec(name=name, opcode=0, uops=lower(spec, ver=ver),
                              rd1_en=True).sha(ver)
    op = DveOp(name, spec, subdim=False, uops_sha=shas)
    dve_ops.OPS.append(op)
    dve_ops._SUB_OPCODE_FOR_NAME[name] = (dve_ops._CUSTOM_DVE_ROW_BASE
                                          + len(dve_ops.OPS) - 1)
    dve_ops.CUSTOM_DVE_SPECS[name] = op.spec
    return op


EXP_OP = _register_exp_op()


def build_nc(S=S_FULL, upto=3):
    """Build the per-core Bass program (same program for all 8 cores)."""
    nc = bacc.Bacc("TRN2", target_bir_lowering=False, debug=False,
                   dynamic_dma_scratch_size=2048)

    xqT = nc.dram_tensor("xqT", [D, S], F32, kind="ExternalInput").ap()
    xkT = nc.dram_tensor("xkT", [D, S], F32, kind="ExternalInput").ap()
    xvT = nc.dram_tensor("xvT", [D, S], F32, kind="ExternalInput").ap()
    wq = nc.dram_tensor("wq", [D, HH], F32, kind="ExternalInput").ap()
    wk = nc.dram_tensor("wk", [D, HH], F32, kind="ExternalInput").ap()
    wv = nc.dram_tensor("wv", [D, HH], F32, kind="ExternalInput").ap()
    wo = nc.dram_tensor("wo", [HH, D], F32, kind="ExternalInput").ap()
    bq = nc.dram_tensor("bq", [HH], F32, kind="ExternalInput").ap()
    bk = nc.dram_tensor("bk", [HH], F32, kind="ExternalInput").ap()
    bv = nc.dram_tensor("bv", [HH], F32, kind="ExternalInput").ap()
    out = nc.dram_tensor("out", [S, D], F32, kind="ExternalOutput").ap()

    NT = S // 512        # 512-token chunks
    NSK = S // 128       # 128-token key tiles
    NKT = D // 128       # 128-wide d_model tiles
    NKB = HH // 128      # 128-wide hidden tiles (also head pairs)

    with tile.TileContext(nc) as tc:
        from contextlib import ExitStack

        with ExitStack() as ctx:
            persist = ctx.enter_context(tc.tile_pool(name="persist", bufs=1))
            qt_sb = persist.tile([128, NKB, S], F32R, tag="qt")
            kt_sb = persist.tile([128, NKB, S], F32R, tag="kt")
            vb_sb = persist.tile([128, NSK, HH], BF16, tag="vb")
            zt_sb = persist.tile([128, NKB, S], F32R, tag="zt")
            wo_sb = persist.tile([128, NKB, D], F32R, tag="wo")
            bq_sb = persist.tile([128, NKB], F32, tag="bq")
            bk_sb = persist.tile([128, NKB], F32, tag="bk")
            bvb_sb = persist.tile([128, HH], F32, tag="bvb")
            c3_sb = persist.tile([128, 1], F32, tag="c3")
            eb_sb = persist.tile([128, 1], F32, tag="eb")

            nc.sync.dma_start(out=bq_sb, in_=bq.rearrange("(kb p) -> p kb", p=128))
            nc.sync.dma_start(out=bk_sb, in_=bk.rearrange("(kb p) -> p kb", p=128))
            bv_bcast_in = bass.AP(tensor=bv.tensor, offset=bv.offset,
                                  ap=[[0, 128], [1, HH]])
            nc.sync.dma_start(out=bvb_sb, in_=bv_bcast_in)
            nc.vector.memset(c3_sb, C3VAL)
            nc.vector.memset(eb_sb, EXP_BIAS)

            # ---------------- phase 1: projections ----------------
            with ExitStack() as c1:
                wpool = c1.enter_context(tc.tile_pool(name="wpool", bufs=2))
                xpool = c1.enter_context(tc.tile_pool(name="xpool", bufs=3))
                p1 = c1.enter_context(tc.tile_pool(name="p1", bufs=4, space="PSUM"))

                # K and Q passes: produce KT/QT [hidden-on-partition, tokens].
                for (xT, w_dram, dst, bias) in ((xkT, wk, kt_sb, bk_sb),
                                                (xqT, wq, qt_sb, bq_sb)):
                    w_sb = wpool.tile([128, NKT, HH], F32R, tag="w",
                                      name=f"w_{dst.name}")
                    nc.sync.dma_start(out=w_sb,
                                      in_=w_dram.bitcast(F32R).rearrange("(kt p) n -> p kt n", p=128))
                    for t in range(NT):
                        xt = xpool.tile([128, NKT, 512], F32R, tag="xt")
                        nc.sync.dma_start(
                            out=xt,
                            in_=xT.bitcast(F32R).rearrange("(kt p) s -> p kt s", p=128)[:, :, t * 512:(t + 1) * 512])
                        for kb in range(NKB):
                            ps = p1.tile([128, 512], F32, tag="ps1")
                            for kt in range(NKT):
                                nc.tensor.matmul(
                                    ps,
                                    lhsT=w_sb[:, kt, kb * 128:(kb + 1) * 128],
                                    rhs=xt[:, kt, :],
                                    start=(kt == 0), stop=(kt == NKT - 1))
                            nc.vector.tensor_scalar_add(
                                dst[:, kb, t * 512:(t + 1) * 512], ps,
                                bias[:, kb:kb + 1])

                # V pass: natural [tokens, hidden], bf16
                wv_sb = wpool.tile([128, NKT, HH], F32R, tag="w", name="w_v")
                nc.sync.dma_start(out=wv_sb,
                                  in_=wv.bitcast(F32R).rearrange("(kt p) n -> p kt n", p=128))
                for t in range(NT):
                    xt = xpool.tile([128, NKT, 512], F32R, tag="xt")
                    nc.sync.dma_start(
                        out=xt,
                        in_=xvT.bitcast(F32R).rearrange("(kt p) s -> p kt s", p=128)[:, :, t * 512:(t + 1) * 512])
                    for m in range(4):
                        ps = p1.tile([128, 512], F32, tag="ps1")
                        for kt in range(NKT):
                            nc.tensor.matmul(
                                ps,
                                lhsT=xt[:, kt, m * 128:(m + 1) * 128],
                                rhs=wv_sb[:, kt, :],
                                start=(kt == 0), stop=(kt == NKT - 1))
                        sk = t * 4 + m
                        nc.vector.tensor_add(vb_sb[:, sk, :], ps, bvb_sb)

            if upto == 1:
                fill = persist.tile([128, D], F32, tag="fill")
                nc.vector.memset(fill, 0.0)
                for t in range(S // 128):
                    nc.sync.dma_start(out=out[t * 128:(t + 1) * 128, :], in_=fill)
            if upto >= 2:
                # ---------------- phase 2: attention ----------------
                from concourse import bass_isa
                with ExitStack() as c2:
                    ptpool = c2.enter_context(tc.tile_pool(name="ptpool", bufs=2))
                    spool = c2.enter_context(tc.tile_pool(name="spool", bufs=1, space="PSUM"))
                    zpool = c2.enter_context(tc.tile_pool(name="zpool", bufs=2, space="PSUM"))
                    lpool = c2.enter_context(tc.tile_pool(name="lpool", bufs=1))
                    npool = c2.enter_context(tc.tile_pool(name="npool", bufs=1))

                    for hp in range(NKB):
                        for c in range(NT):
                            zps = zpool.tile([128, 512], F32, tag="z",
                                             name=f"zps_{hp}_{c}")
                            pts = [ptpool.tile([128, NSK, 512], BF16, tag=f"pt{d}",
                                               name=f"pts{d}_{hp}_{c}")
                                   for d in range(2)]
                            for g0 in range(0, NSK, 3):
                                gs = min(3, NSK - g0)
                                gi = g0 // 3
                                for d in range(2):
                                    sp = spool.tile([128, 3, 512], F32, tag=f"s{d}")
                                    for j in range(gs):
                                        sk = g0 + j
                                        nc.tensor.matmul(
                                            sp[:, j, :],
                                            lhsT=kt_sb[d * 64:(d + 1) * 64, hp,
                                                       sk * 128:(sk + 1) * 128],
                                            rhs=qt_sb[d * 64:(d + 1) * 64, hp,
                                                      c * 512:(c + 1) * 512],
                                            start=True, stop=True)
                                    if gi in DVE_EXP_GROUPS:
                                        nc.vector._custom_dve(
                                            EXP_OP,
                                            out=pts[d][:, g0:g0 + gs, :].bitcast(I16),
                                            in0=sp[:, :gs, :], in1=c3_sb,
                                            s0=MAGIC, s1=A2 / 128.0, imm2=A1)
                                    else:
                                        nc.scalar.activation(
                                            pts[d][:, g0:g0 + gs, :], sp[:, :gs, :],
                                            ACT.Exp, scale=EXP_SCALE, bias=eb_sb)
                                for d in range(2):
                                    h = 2 * hp + d
                                    for j in range(gs):
                                        sk = g0 + j
                                        nc.tensor.matmul(
                                            zps[d * 64:(d + 1) * 64, :],
                                            lhsT=vb_sb[:, sk, h * HD:(h + 1) * HD],
                                            rhs=pts[d][:, sk, :],
                                            start=(sk == 0), stop=(sk == NSK - 1),
                                            tile_position=(0, d * 64))
                            # denominator: tree-sum probs over the 16 sk tiles
                            # into l1[:, d, 0]; partition all-reduce; reciprocal;
                            # scale z during PSUM eviction.
                            l1 = lpool.tile([128, 2, 4, 512], BF16, tag="l1",
                                            name=f"l1_{hp}_{c}")
                            for d in range(2):
                                p3d = pts[d]
                                ev = p3d.rearrange("p (a b) q -> p a b q", b=2)
                                nc.gpsimd.tensor_add(
                                    l1[:, d], ev[:, 0:4, 0, :], ev[:, 0:4, 1, :])
                                nc.gpsimd.tensor_add(
                                    l1[:, d], l1[:, d], ev[:, 4:8, 0, :])
                                nc.gpsimd.tensor_add(
                                    l1[:, d], l1[:, d], ev[:, 4:8, 1, :])
                                nc.vector.tensor_add(l1[:, d, 0], l1[:, d, 0],
                                                     l1[:, d, 1])
                                nc.vector.tensor_add(l1[:, d, 2], l1[:, d, 2],
                                                     l1[:, d, 3])
                                nc.vector.tensor_add(l1[:, d, 0], l1[:, d, 0],
                                                     l1[:, d, 2])
                            ar = npool.tile([128, 2, 512], F32, tag="ar")
                            for d in range(2):
                                nc.gpsimd.partition_all_reduce(
                                    ar[:, d], l1[:, d, 0], channels=128,
                                    reduce_op=bass_isa.ReduceOp.add)
                            rcv = npool.tile([128, 2, 512], F32, tag="rcv")
                            nc.vector.reciprocal_approx_fast(rcv, ar)
                            nc.vector.tensor_mul(
                                zt_sb[0:64, hp, c * 512:(c + 1) * 512],
                                zps[0:64, :], rcv[0:64, 0, :])
                            nc.vector.tensor_mul(
                                zt_sb[64:128, hp, c * 512:(c + 1) * 512],
                                zps[64:128, :], rcv[64:128, 1, :])

            if upto >= 3:
                # ---------------- phase 3: output projection ----------------
                nc.sync.dma_start(out=wo_sb, in_=wo.bitcast(F32R).rearrange("(hb p) n -> p hb n", p=128))
                with ExitStack() as c3:
                    opool = c3.enter_context(tc.tile_pool(name="opool", bufs=3))
                    p3 = c3.enter_context(tc.tile_pool(name="p3", bufs=3, space="PSUM"))
                    for t in range(S // 128):
                        os_t = opool.tile([128, D], F32, tag="os")
                        for n in range(D // 512):
                            po = p3.tile([128, 512], F32, tag="po")
                            for hb in range(NKB):
                                nc.tensor.matmul(
                                    po,
                                    lhsT=zt_sb[:, hb, t * 128:(t + 1) * 128],
                                    rhs=wo_sb[:, hb, n * 512:(n + 1) * 512],
                                    start=(hb == 0), stop=(hb == NKB - 1))
                            nc.vector.tensor_copy(os_t[:, n * 512:(n + 1) * 512], po)
                        nc.sync.dma_start(out=out[t * 128:(t + 1) * 128, :], in_=os_t)

    nc.compile()
    return nc


_NC_CACHE = {}


def _get_nc(S=S_FULL):
    if S not in _NC_CACHE:
        _NC_CACHE[S] = build_nc(S)
    return _NC_CACHE[S]


def make_in_maps(query, key, value, Wq, bq, Wk, bk, Wv, bv, Wo, bo):
    """Shard full inputs into 8 per-core input dicts."""
    f32 = lambda a: np.ascontiguousarray(np.asarray(a, dtype=np.float32))
    in_maps = []
    for core in range(N_CORES):
        b, hg = core // 2, core % 2
        sl = slice(hg * HH, (hg + 1) * HH)
        in_maps.append({
            "xqT": f32(np.asarray(query)[b].T),
            "xkT": f32(np.asarray(key)[b].T),
            "xvT": f32(np.asarray(value)[b].T),
            "wq": f32(np.asarray(Wq)[:, sl]),
            "wk": f32(np.asarray(Wk)[:, sl] * KSC),
            "wv": f32(np.asarray(Wv)[:, sl]),
            "wo": f32(np.asarray(Wo)[sl, :]),
            "bq": f32(np.asarray(bq)[sl]),
            "bk": f32(np.asarray(bk)[sl] * KSC),
            "bv": f32(np.asarray(bv)[sl]),
        })
    return in_maps


def kernel(query, key, value, Wq, bq, Wk, bk, Wv, bv, Wo, bo, **run_kwargs):
    nc = _get_nc(S_FULL)
    in_maps = make_in_maps(query, key, value, Wq, bq, Wk, bk, Wv, bv, Wo, bo)
    res = run_bass_kernel_spmd(nc, in_maps, core_ids=list(range(N_CORES)),
                               **run_kwargs)
    bo_np = np.asarray(bo, dtype=np.float32)
    outs = [np.asarray(r["out"], dtype=np.float32) for r in res.results]
    full = np.stack([outs[2 * b] + outs[2 * b + 1] + bo_np for b in range(B)])
    return full.astype(np.float32)


# revision 6
# speedup vs baseline: 1.0715x; 1.0715x over previous
"""Multi-head attention (B=4, S=2048, D=1024, 16 heads x 64) on 8 trn2 cores.

Sharding: core c handles batch b = c//2 and head-group hg = c%2 (8 heads each,
i.e. columns hg*512:(hg+1)*512 of Wq/Wk/Wv and rows of Wo).  Each core returns
a partial output [S, D]; the host sums the two partials per batch and adds bo.

Per-core kernel (everything "T" = feature-on-partition layout):
  phase 1: QT = (Wq.T @ Xq.T)+bq  [512, S],  KT likewise, V = Xv@Wv+bv [S, 520]
           (V stored in 65-wide head groups: 64 value cols + a ones column).
           Inputs arrive host-transposed as xT [1024, S] so the contraction
           dim (d_model) is already on partitions; matmuls run in float32r
           (full fp32 numerics, full PE rate at free-dim >= 256).
  phase 2: per head-pair hp and 512-col query chunk c:
           scoresT[sk,q] = KT_h.T @ QT_h via row-paired (64+64) matmuls,
           exp fused into the PSUM->SBUF eviction on ScalarE (scale=1/8),
           probsT stored bf16.  zT~[65, 512] = V~_h.T @ probsT accumulated
           over the 16 key tiles; row 64 is the softmax denominator (ones
           column).  Normalize: reciprocal (DVE) -> partition-broadcast (DMA)
           -> multiply during PSUM eviction -> ZT [512, S] fp32.
  phase 3: out = Z @ Wo, natural [S, D] layout, DMA'd to DRAM.
"""

import numpy as np

import concourse.bass as bass
import concourse.tile as tile
from concourse import bacc, mybir
from concourse.bass_utils import run_bass_kernel_spmd

F32 = mybir.dt.float32
F32R = mybir.dt.float32r
BF16 = mybir.dt.bfloat16
ACT = mybir.ActivationFunctionType

D = 1024          # d_model
HH = 512          # heads-per-core * head_dim = 8 * 64
HD = 64           # head dim
NHL = 8           # heads per core
B, S_FULL = 4, 2048
N_CORES = 8


def build_nc(S=S_FULL, debug_taps=False, upto=3):
    """Build the per-core Bass program (same program for all 8 cores)."""
    nc = bacc.Bacc("TRN2", target_bir_lowering=False, debug=False,
                   dynamic_dma_scratch_size=2048)

    xqT = nc.dram_tensor("xqT", [D, S], F32, kind="ExternalInput").ap()
    xkT = nc.dram_tensor("xkT", [D, S], F32, kind="ExternalInput").ap()
    xvT = nc.dram_tensor("xvT", [D, S], F32, kind="ExternalInput").ap()
    wq = nc.dram_tensor("wq", [D, HH], F32, kind="ExternalInput").ap()
    wk = nc.dram_tensor("wk", [D, HH], F32, kind="ExternalInput").ap()
    wv = nc.dram_tensor("wv", [D, HH], F32, kind="ExternalInput").ap()
    wo = nc.dram_tensor("wo", [HH, D], F32, kind="ExternalInput").ap()
    bq = nc.dram_tensor("bq", [HH], F32, kind="ExternalInput").ap()
    bk = nc.dram_tensor("bk", [HH], F32, kind="ExternalInput").ap()
    bv = nc.dram_tensor("bv", [HH], F32, kind="ExternalInput").ap()
    out = nc.dram_tensor("out", [S, D], F32, kind="ExternalOutput").ap()

    NT = S // 512        # 512-token chunks
    NSK = S // 128       # 128-token key tiles
    NKT = D // 128       # 128-wide d_model tiles
    NKB = HH // 128      # 128-wide hidden tiles (also head pairs)

    with tile.TileContext(nc) as tc:
        from contextlib import ExitStack

        with ExitStack() as ctx:
            persist = ctx.enter_context(tc.tile_pool(name="persist", bufs=1))
            qt_sb = persist.tile([128, NKB, S], F32R, tag="qt")
            kt_sb = persist.tile([128, NKB, S], F32R, tag="kt")
            vb_sb = persist.tile([128, NSK, NHL * (HD + 1)], BF16, tag="vb")
            zt_sb = persist.tile([128, NKB, S], F32R, tag="zt")
            wo_sb = persist.tile([128, NKB, D], F32R, tag="wo")
            bq_sb = persist.tile([128, NKB], F32, tag="bq")
            bk_sb = persist.tile([128, NKB], F32, tag="bk")
            bvb_sb = persist.tile([128, HH], F32, tag="bvb")

            nc.sync.dma_start(out=bq_sb, in_=bq.rearrange("(kb p) -> p kb", p=128))
            nc.sync.dma_start(out=bk_sb, in_=bk.rearrange("(kb p) -> p kb", p=128))
            bv_bcast_in = bass.AP(tensor=bv.tensor, offset=bv.offset,
                                  ap=[[0, 128], [1, HH]])
            nc.sync.dma_start(out=bvb_sb, in_=bv_bcast_in)
            # ones columns of V~ (softmax denominator trick)
            ones_view = vb_sb.rearrange("p s (h dd) -> p s h dd", dd=HD + 1)[:, :, :, HD:HD + 1]
            nc.vector.memset(ones_view, 1.0)

            # ---------------- phase 1: projections ----------------
            with ExitStack() as c1:
                wpool = c1.enter_context(tc.tile_pool(name="wpool", bufs=2))
                xpool = c1.enter_context(tc.tile_pool(name="xpool", bufs=3))
                p1 = c1.enter_context(tc.tile_pool(name="p1", bufs=4, space="PSUM"))

                # K and Q passes: produce KT/QT [hidden-on-partition, tokens].
                # K first: scores for query-chunk c need full KT but only
                # chunk c of QT, so attention starts while Q still streams.
                for (xT, w_dram, dst, bias) in ((xkT, wk, kt_sb, bk_sb),
                                                (xqT, wq, qt_sb, bq_sb)):
                    w_sb = wpool.tile([128, NKT, HH], F32R, tag="w",
                                      name=f"w_{dst.name}")
                    nc.sync.dma_start(out=w_sb,
                                      in_=w_dram.bitcast(F32R).rearrange("(kt p) n -> p kt n", p=128))
                    for t in range(NT):
                        xt = xpool.tile([128, NKT, 512], F32R, tag="xt")
                        nc.sync.dma_start(
                            out=xt,
                            in_=xT.bitcast(F32R).rearrange("(kt p) s -> p kt s", p=128)[:, :, t * 512:(t + 1) * 512])
                        for kb in range(NKB):
                            ps = p1.tile([128, 512], F32, tag="ps1")
                            for kt in range(NKT):
                                nc.tensor.matmul(
                                    ps,
                                    lhsT=w_sb[:, kt, kb * 128:(kb + 1) * 128],
                                    rhs=xt[:, kt, :],
                                    start=(kt == 0), stop=(kt == NKT - 1))
                            nc.vector.tensor_scalar_add(
                                dst[:, kb, t * 512:(t + 1) * 512], ps,
                                bias[:, kb:kb + 1])

                # V pass: natural [tokens, hidden] with 65-wide head groups
                wv_sb = wpool.tile([128, NKT, HH], F32R, tag="w", name="w_v")
                nc.sync.dma_start(out=wv_sb,
                                  in_=wv.bitcast(F32R).rearrange("(kt p) n -> p kt n", p=128))
                for t in range(NT):
                    xt = xpool.tile([128, NKT, 512], F32R, tag="xt")
                    nc.sync.dma_start(
                        out=xt,
                        in_=xvT.bitcast(F32R).rearrange("(kt p) s -> p kt s", p=128)[:, :, t * 512:(t + 1) * 512])
                    for m in range(4):
                        ps = p1.tile([128, 512], F32, tag="ps1")
                        for kt in range(NKT):
                            nc.tensor.matmul(
                                ps,
                                lhsT=xt[:, kt, m * 128:(m + 1) * 128],
                                rhs=wv_sb[:, kt, :],
                                start=(kt == 0), stop=(kt == NKT - 1))
                        sk = t * 4 + m
                        vdst = vb_sb[:, sk, :].rearrange(
                            "p (h dd) -> p h dd", dd=HD + 1)[:, :, 0:HD]
                        nc.vector.tensor_add(
                            vdst,
                            ps.rearrange("p (h d) -> p h d", d=HD),
                            bvb_sb.rearrange("p (h d) -> p h d", d=HD))

            if upto == 1:
                fill = persist.tile([128, D], F32, tag="fill")
                nc.vector.memset(fill, 0.0)
                for t in range(S // 128):
                    nc.sync.dma_start(out=out[t * 128:(t + 1) * 128, :], in_=fill)
            if upto >= 2:
                # ---------------- phase 2: attention ----------------
                with ExitStack() as c2:
                    ptpool = c2.enter_context(tc.tile_pool(name="ptpool", bufs=2))
                    spool = c2.enter_context(tc.tile_pool(name="spool", bufs=1, space="PSUM"))
                    zpool = c2.enter_context(tc.tile_pool(name="zpool", bufs=1, space="PSUM"))
                    rpool = c2.enter_context(tc.tile_pool(name="rpool", bufs=3))

                    for hp in range(NKB):
                        for c in range(NT):
                            zps = [zpool.tile([HD + 1, 512], F32, tag=f"z{d}",
                                              name=f"zps{d}_{hp}_{c}")
                                   for d in range(2)]
                            pts = [ptpool.tile([128, NSK, 512], BF16, tag=f"pt{d}",
                                               name=f"pts{d}_{hp}_{c}")
                                   for d in range(2)]
                            for g0 in range(0, NSK, 3):
                                gs = min(3, NSK - g0)
                                for d in range(2):
                                    sp = spool.tile([128, 3, 512], F32, tag=f"s{d}")
                                    for j in range(gs):
                                        sk = g0 + j
                                        nc.tensor.matmul(
                                            sp[:, j, :],
                                            lhsT=kt_sb[d * 64:(d + 1) * 64, hp,
                                                       sk * 128:(sk + 1) * 128],
                                            rhs=qt_sb[d * 64:(d + 1) * 64, hp,
                                                      c * 512:(c + 1) * 512],
                                            start=True, stop=True)
                                    nc.scalar.activation(
                                        pts[d][:, g0:g0 + gs, :], sp[:, :gs, :],
                                        ACT.Exp, scale=0.125)
                                for d in range(2):
                                    h = 2 * hp + d
                                    for j in range(gs):
                                        sk = g0 + j
                                        nc.tensor.matmul(
                                            zps[d],
                                            lhsT=vb_sb[:, sk, h * (HD + 1):(h + 1) * (HD + 1)],
                                            rhs=pts[d][:, sk, :],
                                            start=(sk == 0), stop=(sk == NSK - 1))
                            for d in range(2):
                                # evict z~ to SBUF immediately so the PSUM bank
                                # frees for the next chunk; normalize from SBUF
                                zr = rpool.tile([HD + 1, 512], F32, tag="zr")
                                nc.vector.tensor_copy(zr, zps[d])
                                # custom-DVE recip can't read base_partition 64:
                                # stage the denominator row at partition 0 first
                                dn = rpool.tile([1, 512], F32, tag="dn")
                                nc.vector.tensor_copy(dn, zr[HD:HD + 1, :])
                                rc = rpool.tile([1, 512], F32, tag="rc")
                                nc.vector.reciprocal_approx_fast(rc, dn)
                                bc = rpool.tile([HD, 512], F32, tag="bc")
                                nc.gpsimd.partition_broadcast(bc, rc, channels=HD)
                                nc.vector.tensor_mul(
                                    zt_sb[d * 64:d * 64 + HD, hp, c * 512:(c + 1) * 512],
                                    zr[0:HD, :], bc)

            if debug_taps:
                NSKl = S // 128
                qt_d = nc.dram_tensor("qt_d", [128, NKB, S], F32, kind="ExternalOutput").ap()
                kt_d = nc.dram_tensor("kt_d", [128, NKB, S], F32, kind="ExternalOutput").ap()
                vb_d = nc.dram_tensor("vb_d", [128, NSKl, NHL * (HD + 1)], F32, kind="ExternalOutput").ap()
                zt_d = nc.dram_tensor("zt_d", [128, NKB, S], F32, kind="ExternalOutput").ap()
                with tc.tile_pool(name="dbg", bufs=1) as dbg:
                    vb_f = dbg.tile([128, NSKl, NHL * (HD + 1)], F32)
                    nc.vector.tensor_copy(vb_f, vb_sb)
                    nc.sync.dma_start(out=vb_d, in_=vb_f)
                nc.sync.dma_start(out=qt_d, in_=qt_sb.bitcast(F32))
                nc.sync.dma_start(out=kt_d, in_=kt_sb.bitcast(F32))
                nc.sync.dma_start(out=zt_d, in_=zt_sb.bitcast(F32))

            if upto >= 3:
                # ---------------- phase 3: output projection ----------------
                # wo loads here (not at kernel start) to keep the early DMA
                # window clear for xkT/xqT, which gate the first scores
                nc.sync.dma_start(out=wo_sb, in_=wo.bitcast(F32R).rearrange("(hb p) n -> p hb n", p=128))
                with ExitStack() as c3:
                    opool = c3.enter_context(tc.tile_pool(name="opool", bufs=3))
                    p3 = c3.enter_context(tc.tile_pool(name="p3", bufs=3, space="PSUM"))
                    for t in range(S // 128):
                        os_t = opool.tile([128, D], F32, tag="os")
                        for n in range(D // 512):
                            po = p3.tile([128, 512], F32, tag="po")
                            for hb in range(NKB):
                                nc.tensor.matmul(
                                    po,
                                    lhsT=zt_sb[:, hb, t * 128:(t + 1) * 128],
                                    rhs=wo_sb[:, hb, n * 512:(n + 1) * 512],
                                    start=(hb == 0), stop=(hb == NKB - 1))
                            nc.vector.tensor_copy(os_t[:, n * 512:(n + 1) * 512], po)
                        nc.sync.dma_start(out=out[t * 128:(t + 1) * 128, :], in_=os_t)

    nc.compile()
    return nc


_NC_CACHE = {}


def _get_nc(S=S_FULL):
    if S not in _NC_CACHE:
        _NC_CACHE[S] = build_nc(S)
    return _NC_CACHE[S]


def make_in_maps(query, key, value, Wq, bq, Wk, bk, Wv, bv, Wo, bo):
    """Shard full inputs into 8 per-core input dicts."""
    f32 = lambda a: np.ascontiguousarray(np.asarray(a, dtype=np.float32))
    in_maps = []
    for core in range(N_CORES):
        b, hg = core // 2, core % 2
        sl = slice(hg * HH, (hg + 1) * HH)
        in_maps.append({
            "xqT": f32(np.asarray(query)[b].T),
            "xkT": f32(np.asarray(key)[b].T),
            "xvT": f32(np.asarray(value)[b].T),
            "wq": f32(np.asarray(Wq)[:, sl]),
            "wk": f32(np.asarray(Wk)[:, sl]),
            "wv": f32(np.asarray(Wv)[:, sl]),
            "wo": f32(np.asarray(Wo)[sl, :]),
            "bq": f32(np.asarray(bq)[sl]),
            "bk": f32(np.asarray(bk)[sl]),
            "bv": f32(np.asarray(bv)[sl]),
        })
    return in_maps


def kernel(query, key, value, Wq, bq, Wk, bk, Wv, bv, Wo, bo, **run_kwargs):
    nc = _get_nc(S_FULL)
    in_maps = make_in_maps(query, key, value, Wq, bq, Wk, bk, Wv, bv, Wo, bo)
    res = run_bass_kernel_spmd(nc, in_maps, core_ids=list(range(N_CORES)),
                               **run_kwargs)
    bo_np = np.asarray(bo, dtype=np.float32)
    outs = [np.asarray(r["out"], dtype=np.float32) for r in res.results]
    full = np.stack([outs[2 * b] + outs[2 * b + 1] + bo_np for b in range(B)])
    return full.astype(np.float32)

